# revision 5
# baseline (speedup 1.0000x reference)
"""DiffuseRouter kernel for 8 TRN2 NeuronCores.

Reference computation (enable_time=False, soft_time_routing=True):
    out[b, l, d] = (1/3) * sum_g sum_e expert_emb_g[e, b, l, d]
i.e. a uniform-weighted sum of 28 expert planes per batch element.

Sharding: pure data-parallel over batch B=8 -> one batch element per core.
No collectives needed (B == n_cores), which is strictly less traffic than
expert-parallel + all-reduce.

Precision: the host quantizes each plane to bf16 with the constant 1/3
granularity weight folded into the quantization scale (the probs are
input-independent: ones/3), so the device computes the pure 28-plane sum
and stores bf16 — 19.0 MB/core of HBM traffic instead of 38.0 MB/core.
Measured relative error ~5.3e-3, well inside the 2e-2 gate.

Performance structure (the kernel is DMA-bandwidth-bound at ~412 GB/s/core):
- planes 0..25 are host-interleaved into 13 pair tiles [128, 2*2560] whose
  partition lines are 10240 B contiguous -> full-size DMA descriptors.
- DVE runs two bf16 chains with alternating ops (RAW distance 2, 2x perf
  mode): chain0 <- planes {0,1} + left pair halves + plane 26, chain1 <-
  planes {2,3} + right pair halves + plane 27.
- planes 26/27 load in quarters: each quarter does (c0 += p26q), then
  (c1 += p27q, out = c0+c1, store) — so only ~0.8 us of DVE work trails
  the last DMA byte, and each quarter store issues immediately on the ACT
  ring while the SP ring finishes the input stream.
"""

import numpy as np
import ml_dtypes

import concourse.bacc as bacc
import concourse.tile as tile
from concourse import mybir
from concourse.alu_op_type import AluOpType
from concourse.bass_utils import run_bass_kernel_spmd

N_CORES = 8
E_TOTAL = 28  # 4 + 8 + 16 experts across the 3 granularity levels
L, D = 256, 1280
P = 128  # SBUF partitions
FD = (L // P) * D  # 2560 free-dim elements per partition per plane
N_PAIRS = 13  # planes 0..25 as pair tiles; 26 and 27 quartered
SCALE = 1.0 / 3.0
BF16 = ml_dtypes.bfloat16

_NC_CACHE = None


def _build_nc():
    nc = bacc.Bacc(
        "TRN2", target_bir_lowering=False, debug=False, enable_partition_id=False
    )
    xp = nc.dram_tensor(
        "xp", [N_PAIRS, P, 2 * FD], mybir.dt.bfloat16, kind="ExternalInput"
    )
    x26 = nc.dram_tensor("x26", [P, FD], mybir.dt.bfloat16, kind="ExternalInput")
    x27 = nc.dram_tensor("x27", [P, FD], mybir.dt.bfloat16, kind="ExternalInput")
    out = nc.dram_tensor("out", [L, D], mybir.dt.bfloat16, kind="ExternalOutput")

    xp_t = xp.ap()
    out_t = out.ap().rearrange("(p a) d -> p (a d)", a=2)

    add = AluOpType.add
    LEFT = slice(0, FD)
    RIGHT = slice(FD, 2 * FD)

    with tile.TileContext(nc) as tc:
        with (
            tc.tile_pool(name="in", bufs=8) as pin,
            tc.tile_pool(name="acc", bufs=2) as pacc,
            tc.tile_pool(name="outp", bufs=1) as pout,
        ):
            accs = [
                pacc.tile([P, FD], mybir.dt.bfloat16, name=f"acc{i}", tag=f"acc{i}")
                for i in range(2)
            ]
            obuf = pout.tile([P, FD], mybir.dt.bfloat16, name="obuf", tag="obuf")

            # Pair tiles 0 and 1 initialize the two chains.
            for k in range(2):
                t = pin.tile([P, 2 * FD], mybir.dt.bfloat16)
                nc.sync.dma_start(out=t[:], in_=xp_t[k])
                nc.vector.tensor_tensor(accs[k][:], t[:, LEFT], t[:, RIGHT], add)
            # Pairs 2..12: left half -> chain0, right half -> chain1.
            for k in range(2, N_PAIRS):
                t = pin.tile([P, 2 * FD], mybir.dt.bfloat16)
                nc.sync.dma_start(out=t[:], in_=xp_t[k])
                nc.vector.tensor_tensor(accs[0][:], accs[0][:], t[:, LEFT], add)
                nc.vector.tensor_tensor(accs[1][:], accs[1][:], t[:, RIGHT], add)
            # Plane 26 quarters -> chain0 (loads drain before plane 27's).
            Q = FD // 4
            for qi in range(4):
                q = slice(qi * Q, (qi + 1) * Q)
                qt = pin.tile(
                    [P, Q], mybir.dt.bfloat16, name=f"t26q{qi}", tag=f"t26q{qi}"
                )
                nc.sync.dma_start(out=qt[:], in_=x26.ap()[:, q])
                nc.vector.tensor_tensor(accs[0][:, q], accs[0][:, q], qt[:], add)
            # Plane 27 quarters -> chain1, then merge into obuf and store.
            for qi in range(4):
                q = slice(qi * Q, (qi + 1) * Q)
                qt = pin.tile(
                    [P, Q], mybir.dt.bfloat16, name=f"t27q{qi}", tag=f"t27q{qi}"
                )
                nc.sync.dma_start(out=qt[:], in_=x27.ap()[:, q])
                nc.vector.tensor_tensor(accs[1][:, q], accs[1][:, q], qt[:], add)
                nc.vector.tensor_tensor(obuf[:, q], accs[0][:, q], accs[1][:, q], add)
                nc.scalar.dma_start(out=out_t[:, q], in_=obuf[:, q])
    nc.compile()
    return nc


def _get_nc():
    global _NC_CACHE
    if _NC_CACHE is None:
        _NC_CACHE = _build_nc()
    return _NC_CACHE


def _run(inputs, trace=False, trace_kwargs=None):
    e0 = np.asarray(inputs["expert_emb_0"])
    e1 = np.asarray(inputs["expert_emb_1"])
    e2 = np.asarray(inputs["expert_emb_2"])
    B = e0.shape[1]
    assert B == N_CORES, f"expected B == {N_CORES}, got {B}"

    s = np.float32(SCALE)
    eb0 = (e0 * s).astype(BF16)
    eb1 = (e1 * s).astype(BF16)
    eb2 = (e2 * s).astype(BF16)

    in_maps = []
    for b in range(B):
        xb = np.concatenate([eb0[:, b], eb1[:, b], eb2[:, b]], axis=0)
        # [28, 256, 1280] -> partition lines: [28, 128, 2560]
        xl = xb.reshape(E_TOTAL, P, FD)
        # Pair-interleave planes 0..25: [13, 128, 2, 2560] so each partition
        # line of a pair tile is 10240 B contiguous.
        pairs = np.ascontiguousarray(
            xl[: 2 * N_PAIRS].reshape(N_PAIRS, 2, P, FD).transpose(0, 2, 1, 3)
        ).reshape(N_PAIRS, P, 2 * FD)
        in_maps.append(
            {
                "xp": pairs,
                "x26": np.ascontiguousarray(xl[26]),
                "x27": np.ascontiguousarray(xl[27]),
            }
        )

    kw = {}
    if trace:
        kw["trace"] = True
        if trace_kwargs:
            kw.update(trace_kwargs)
    try:
        res = run_bass_kernel_spmd(_get_nc(), in_maps, list(range(N_CORES)), **kw)
    except Exception:
        # One retry: transient device errors (e.g. NRT unrecoverable after a
        # prior wedged run) usually clear on re-dispatch.
        res = run_bass_kernel_spmd(_get_nc(), in_maps, list(range(N_CORES)), **kw)
    out = np.stack([res.results[b]["out"] for b in range(B)], axis=0)
    return out.astype(np.float32), res


def kernel(**inputs) -> np.ndarray:
    out, _ = _run(inputs, trace=False)
    return out


# revision 7
# speedup vs baseline: 1.0932x; 1.0932x over previous
"""DiffuseRouter kernel for 8 TRN2 NeuronCores.

Reference computation (enable_time=False, soft_time_routing=True):
    out[b, l, d] = (1/3) * sum_g sum_e expert_emb_g[e, b, l, d]
i.e. a uniform-weighted sum of 28 expert planes per batch element.

Sharding: pure data-parallel over batch B=8 -> one batch element per core.
No collectives needed (B == n_cores), which is strictly less traffic than
expert-parallel + all-reduce.

Precision: the host quantizes each plane to bf16 with the constant 1/3
granularity weight folded into the quantization scale (the probs are
input-independent: ones/3), so the device computes the pure 28-plane sum
and stores bf16 — 19.0 MB/core of HBM traffic instead of 38.0 MB/core.
Measured relative error ~5.3e-3, well inside the 2e-2 gate.

Performance structure (the kernel is DMA-bandwidth-bound at ~412 GB/s/core):
- planes 0..25 are host-interleaved into 13 pair tiles [128, 2*2560] whose
  partition lines are 10240 B contiguous -> full-size DMA descriptors.
- DVE runs two bf16 chains with alternating ops (RAW distance 2, 2x perf
  mode): chain0 <- planes {0,1} + left pair halves + plane 26, chain1 <-
  planes {2,3} + right pair halves + plane 27.
- planes 26/27 load in quarters: each quarter does (c0 += p26q), then
  (c1 += p27q, out = c0+c1, store) — so only ~0.8 us of DVE work trails
  the last DMA byte, and each quarter store issues immediately on the ACT
  ring while the SP ring finishes the input stream.
"""

import numpy as np
import ml_dtypes

import concourse.bacc as bacc
import concourse.tile as tile
from concourse import mybir
from concourse.alu_op_type import AluOpType
from concourse.bass_utils import run_bass_kernel_spmd

N_CORES = 8
E_TOTAL = 28  # 4 + 8 + 16 experts across the 3 granularity levels
L, D = 256, 1280
P = 128  # SBUF partitions
FD = (L // P) * D  # 2560 free-dim elements per partition per plane
N_PAIRS = 13  # planes 0..25 as pair tiles; 26 and 27 quartered
SCALE = 1.0 / 3.0
BF16 = ml_dtypes.bfloat16

_NC_CACHE = None


def _build_nc():
    nc = bacc.Bacc(
        "TRN2", target_bir_lowering=False, debug=False, enable_partition_id=False
    )
    xp = nc.dram_tensor(
        "xp", [N_PAIRS, P, 2 * FD], mybir.dt.bfloat16, kind="ExternalInput"
    )
    x26 = nc.dram_tensor("x26", [P, FD], mybir.dt.bfloat16, kind="ExternalInput")
    x27 = nc.dram_tensor("x27", [P, FD], mybir.dt.bfloat16, kind="ExternalInput")
    out = nc.dram_tensor("out", [L, D], mybir.dt.bfloat16, kind="ExternalOutput")

    xp_t = xp.ap()
    out_t = out.ap().rearrange("(p a) d -> p (a d)", a=2)

    add = AluOpType.add
    LEFT = slice(0, FD)
    RIGHT = slice(FD, 2 * FD)

    with tile.TileContext(nc) as tc:
        with (
            tc.tile_pool(name="in", bufs=8) as pin,
            tc.tile_pool(name="acc", bufs=2) as pacc,
            tc.tile_pool(name="outp", bufs=1) as pout,
        ):
            accs = [
                pacc.tile([P, FD], mybir.dt.bfloat16, name=f"acc{i}", tag=f"acc{i}")
                for i in range(2)
            ]
            obuf = pout.tile([P, FD], mybir.dt.bfloat16, name="obuf", tag="obuf")

            # Pair tiles 0 and 1 initialize the two chains.
            for k in range(2):
                t = pin.tile([P, 2 * FD], mybir.dt.bfloat16)
                nc.sync.dma_start(out=t[:], in_=xp_t[k])
                nc.vector.tensor_tensor(accs[k][:], t[:, LEFT], t[:, RIGHT], add)
            # Pairs 2..12: left half -> chain0, right half -> chain1.
            for k in range(2, N_PAIRS):
                t = pin.tile([P, 2 * FD], mybir.dt.bfloat16)
                nc.sync.dma_start(out=t[:], in_=xp_t[k])
                nc.vector.tensor_tensor(accs[0][:], accs[0][:], t[:, LEFT], add)
                nc.vector.tensor_tensor(accs[1][:], accs[1][:], t[:, RIGHT], add)
            # Plane 26 quarters -> chain0 (loads drain before plane 27's).
            Q = FD // 4
            for qi in range(4):
                q = slice(qi * Q, (qi + 1) * Q)
                qt = pin.tile(
                    [P, Q], mybir.dt.bfloat16, name=f"t26q{qi}", tag=f"t26q{qi}", bufs=1
                )
                nc.sync.dma_start(out=qt[:], in_=x26.ap()[:, q])
                nc.vector.tensor_tensor(accs[0][:, q], accs[0][:, q], qt[:], add)
            # Plane 27 quarters -> chain1, then merge into obuf and store.
            for qi in range(4):
                q = slice(qi * Q, (qi + 1) * Q)
                qt = pin.tile(
                    [P, Q], mybir.dt.bfloat16, name=f"t27q{qi}", tag=f"t27q{qi}", bufs=1
                )
                nc.sync.dma_start(out=qt[:], in_=x27.ap()[:, q])
                nc.vector.tensor_tensor(accs[1][:, q], accs[1][:, q], qt[:], add)
                nc.vector.tensor_tensor(obuf[:, q], accs[0][:, q], accs[1][:, q], add)
                nc.scalar.dma_start(out=out_t[:, q], in_=obuf[:, q])
    nc.compile()
    return nc


def _get_nc():
    global _NC_CACHE
    if _NC_CACHE is None:
        _NC_CACHE = _build_nc()
    return _NC_CACHE


def _run(inputs, trace=False, trace_kwargs=None):
    e0 = np.asarray(inputs["expert_emb_0"])
    e1 = np.asarray(inputs["expert_emb_1"])
    e2 = np.asarray(inputs["expert_emb_2"])
    B = e0.shape[1]
    assert B == N_CORES, f"expected B == {N_CORES}, got {B}"

    s = np.float32(SCALE)
    eb0 = (e0 * s).astype(BF16)
    eb1 = (e1 * s).astype(BF16)
    eb2 = (e2 * s).astype(BF16)

    in_maps = []
    for b in range(B):
        xb = np.concatenate([eb0[:, b], eb1[:, b], eb2[:, b]], axis=0)
        # [28, 256, 1280] -> partition lines: [28, 128, 2560]
        xl = xb.reshape(E_TOTAL, P, FD)
        # Pair-interleave planes 0..25: [13, 128, 2, 2560] so each partition
        # line of a pair tile is 10240 B contiguous.
        pairs = np.ascontiguousarray(
            xl[: 2 * N_PAIRS].reshape(N_PAIRS, 2, P, FD).transpose(0, 2, 1, 3)
        ).reshape(N_PAIRS, P, 2 * FD)
        in_maps.append(
            {
                "xp": pairs,
                "x26": np.ascontiguousarray(xl[26]),
                "x27": np.ascontiguousarray(xl[27]),
            }
        )

    kw = {}
    if trace:
        kw["trace"] = True
        if trace_kwargs:
            kw.update(trace_kwargs)
    try:
        res = run_bass_kernel_spmd(_get_nc(), in_maps, list(range(N_CORES)), **kw)
    except Exception:
        # One retry: transient device errors (e.g. NRT unrecoverable after a
        # prior wedged run) usually clear on re-dispatch.
        res = run_bass_kernel_spmd(_get_nc(), in_maps, list(range(N_CORES)), **kw)
    out = np.stack([res.results[b]["out"] for b in range(B)], axis=0)
    return out.astype(np.float32), res


def kernel(**inputs) -> np.ndarray:
    out, _ = _run(inputs, trace=False)
    return out


# revision 12
# speedup vs baseline: 1.1013x; 1.0074x over previous
"""DiffuseRouter kernel for 8 TRN2 NeuronCores.

Reference computation (enable_time=False, soft_time_routing=True):
    out[b, l, d] = (1/3) * sum_g sum_e expert_emb_g[e, b, l, d]
i.e. a uniform-weighted sum of 28 expert planes per batch element.

Sharding: pure data-parallel over batch B=8 -> one batch element per core.
No collectives needed (B == n_cores), which is strictly less traffic than
expert-parallel + all-reduce.

Precision: the host quantizes each plane to bf16 with the constant 1/3
granularity weight folded into the quantization scale (the probs are
input-independent: ones/3), so the device computes the pure 28-plane sum
and stores bf16 — 19.0 MB/core of HBM traffic instead of 38.0 MB/core.
Measured relative error ~5.3e-3, well inside the 2e-2 gate.

Performance structure (the kernel is DMA-bandwidth-bound, ~410 GB/s/core):
- planes 0..23 are host-interleaved into 12 pair tiles [128, 2*2560] whose
  partition lines are 10240 B contiguous -> full-size DMA descriptors.
- DVE accumulates two bf16 chains with alternating ops (RAW distance 2,
  2x perf mode); DVE's per-pair time (~2.9 us) is under the pair load
  time (~3.2 us), so the stream stays DMA-bound.
- the tile sizes taper at the end (pairs -> single planes 24,25 ->
  quarter-interleaved mini pair tiles of planes 26|27) so the DVE backlog
  at end-of-stream shrinks: each mini tile does (c0q += p26q,
  c1q += p27q, outq = c0q+c1q) and its store issues immediately on the
  ACT ring while the SP ring finishes the input stream.
"""

import numpy as np
import ml_dtypes

import concourse.bacc as bacc
import concourse.tile as tile
from concourse import mybir
from concourse.alu_op_type import AluOpType
from concourse.bass_utils import run_bass_kernel_spmd

N_CORES = 8
E_TOTAL = 28  # 4 + 8 + 16 experts across the 3 granularity levels
L, D = 256, 1280
P = 128  # SBUF partitions
FD = (L // P) * D  # 2560 free-dim elements per partition per plane
N_PAIRS = 12  # planes 0..23 as pair tiles
Q = FD // 4  # quarter of a plane's partition line
SCALE = 1.0 / 3.0
BF16 = ml_dtypes.bfloat16

_NC_CACHE = None


def _build_nc():
    nc = bacc.Bacc(
        "TRN2", target_bir_lowering=False, debug=False, enable_partition_id=False
    )
    xp = nc.dram_tensor(
        "xp", [N_PAIRS, P, 2 * FD], mybir.dt.bfloat16, kind="ExternalInput"
    )
    x24 = nc.dram_tensor("x24", [P, FD], mybir.dt.bfloat16, kind="ExternalInput")
    x25 = nc.dram_tensor("x25", [P, FD], mybir.dt.bfloat16, kind="ExternalInput")
    # Mini pair tiles: [4, 128, 2*Q]; mini c = (p26 quarter c | p27 quarter c).
    xm = nc.dram_tensor("xm", [4, P, 2 * Q], mybir.dt.bfloat16, kind="ExternalInput")
    out = nc.dram_tensor("out", [L, D], mybir.dt.bfloat16, kind="ExternalOutput")

    out_t = out.ap().rearrange("(p a) d -> p (a d)", a=2)

    add = AluOpType.add
    LEFT = slice(0, FD)
    RIGHT = slice(FD, 2 * FD)

    with tile.TileContext(nc) as tc:
        with (
            tc.tile_pool(name="in", bufs=8) as pin,
            tc.tile_pool(name="acc", bufs=2) as pacc,
            tc.tile_pool(name="outp", bufs=1) as pout,
        ):
            accs = [
                pacc.tile([P, FD], mybir.dt.bfloat16, name=f"acc{i}", tag=f"acc{i}")
                for i in range(2)
            ]
            obuf = pout.tile([P, FD], mybir.dt.bfloat16, name="obuf", tag="obuf")

            # Pair tiles 0 and 1 initialize the two chains.
            for k in range(2):
                t = pin.tile([P, 2 * FD], mybir.dt.bfloat16)
                nc.sync.dma_start(out=t[:], in_=xp.ap()[k])
                nc.vector.tensor_tensor(accs[k][:], t[:, LEFT], t[:, RIGHT], add)
            # Pairs 2..11: left half -> chain0, right half -> chain1.
            for k in range(2, N_PAIRS):
                t = pin.tile([P, 2 * FD], mybir.dt.bfloat16)
                nc.sync.dma_start(out=t[:], in_=xp.ap()[k])
                nc.vector.tensor_tensor(accs[0][:], accs[0][:], t[:, LEFT], add)
                nc.vector.tensor_tensor(accs[1][:], accs[1][:], t[:, RIGHT], add)
            # Single planes 24 -> chain0, 25 -> chain1.
            for nm, x_t, c in (("t24", x24, 0), ("t25", x25, 1)):
                t = pin.tile([P, FD], mybir.dt.bfloat16, name=nm, tag=nm, bufs=1)
                nc.sync.dma_start(out=t[:], in_=x_t.ap())
                nc.vector.tensor_tensor(accs[c][:], accs[c][:], t[:], add)
            # Mini pair tiles (planes 26|27 quarter-wise): finish + store.
            for qi in range(4):
                q = slice(qi * Q, (qi + 1) * Q)
                t = pin.tile(
                    [P, 2 * Q], mybir.dt.bfloat16, name=f"tm{qi}", tag=f"tm{qi}", bufs=1
                )
                nc.sync.dma_start(out=t[:], in_=xm.ap()[qi])
                nc.vector.tensor_tensor(accs[0][:, q], accs[0][:, q], t[:, :Q], add)
                nc.vector.tensor_tensor(accs[1][:, q], accs[1][:, q], t[:, Q:], add)
                nc.vector.tensor_tensor(obuf[:, q], accs[0][:, q], accs[1][:, q], add)
                # Store each quarter the moment its merge retires, on the ACT
                # HWDGE ring so stores never queue behind input loads (SP).
                nc.scalar.dma_start(out=out_t[:, q], in_=obuf[:, q])
    nc.compile()
    return nc


def _get_nc():
    global _NC_CACHE
    if _NC_CACHE is None:
        _NC_CACHE = _build_nc()
    return _NC_CACHE


def _run(inputs, trace=False, trace_kwargs=None):
    e0 = np.asarray(inputs["expert_emb_0"])
    e1 = np.asarray(inputs["expert_emb_1"])
    e2 = np.asarray(inputs["expert_emb_2"])
    B = e0.shape[1]
    assert B == N_CORES, f"expected B == {N_CORES}, got {B}"

    s = np.float32(SCALE)
    eb0 = (e0 * s).astype(BF16)
    eb1 = (e1 * s).astype(BF16)
    eb2 = (e2 * s).astype(BF16)

    in_maps = []
    for b in range(B):
        xb = np.concatenate([eb0[:, b], eb1[:, b], eb2[:, b]], axis=0)
        # [28, 256, 1280] -> partition lines: [28, 128, 2560]
        xl = xb.reshape(E_TOTAL, P, FD)
        # Pair-interleave planes 0..23 -> 10240 B partition lines.
        pairs = np.ascontiguousarray(
            xl[: 2 * N_PAIRS].reshape(N_PAIRS, 2, P, FD).transpose(0, 2, 1, 3)
        ).reshape(N_PAIRS, P, 2 * FD)
        # Mini pair tiles: [4, 128, 2Q] = (p26 quarter | p27 quarter).
        q2627 = np.stack([xl[26], xl[27]], axis=0).reshape(2, P, 4, Q)
        minis = np.ascontiguousarray(q2627.transpose(2, 1, 0, 3)).reshape(
            4, P, 2 * Q
        )
        in_maps.append(
            {
                "xp": pairs,
                "x24": np.ascontiguousarray(xl[24]),
                "x25": np.ascontiguousarray(xl[25]),
                "xm": minis,
            }
        )

    kw = {}
    if trace:
        kw["trace"] = True
        if trace_kwargs:
            kw.update(trace_kwargs)
    try:
        res = run_bass_kernel_spmd(_get_nc(), in_maps, list(range(N_CORES)), **kw)
    except Exception:
        # One retry: transient device errors (e.g. NRT unrecoverable after a
        # prior wedged run) usually clear on re-dispatch.
        res = run_bass_kernel_spmd(_get_nc(), in_maps, list(range(N_CORES)), **kw)
    out = np.stack([res.results[b]["out"] for b in range(B)], axis=0)
    return out.astype(np.float32), res


def kernel(**inputs) -> np.ndarray:
    out, _ = _run(inputs, trace=False)
    return out
